# revision 45
# baseline (speedup 1.0000x reference)
"""Fused spatial+temporal AttentionBlock kernel for 8 Trainium2 NeuronCores.

Sharding: stage 1 (spatial attention, batch (b t) = 32) data-parallel, 4
batches/core; stage 2 (temporal attention, batch (b h w) = 2048)
data-parallel over hw, 128 positions/core; 4 AllToAll collectives between
stages (first three hidden under stage-1 compute).

Attention uses the scoresT [s, t] orientation: softmax denominator comes from
a ones-column in the AV matmul; exp skips max-subtraction (scores bounded,
fp32-safe, softmax shift-invariant).  Large matmuls run as float32r (1
cycle/row vs 4 for fp32).  GroupNorm rstd is computed on the vector engine
(bit-trick + 2 Newton steps) so ACT only ever needs the exp table set.
Both stage loops are software-pipelined (prefix of iteration k+1 emitted
before body of k) since engines execute their streams in order.
"""

import os
import sys
import threading

sys.path.insert(0, '/opt/trn_rl_repo')
sys.path.insert(0, '/opt/trn_rl_repo/concourse')

import numpy as np

import concourse.bass as bass
import concourse.tile as tile
from concourse import bacc, mybir
from concourse.bass_utils import run_bass_kernel_spmd

F32 = mybir.dt.float32
F32R = mybir.dt.float32r
I32 = mybir.dt.int32
AF = mybir.ActivationFunctionType
OP = mybir.AluOpType
AX = mybir.AxisListType


def _r(ap):
    return ap.bitcast(F32R)


N_CORES = 8
C = 192
B = 2
T = 16
HW = 1024
NB1 = 4
PHW = 128
HEADS = 3
CH = 64
NGRP = 32
GS = 6
EPS = 1e-5
QKSCALE = CH ** -0.25
BIG = 60.0


def _consts():
    G0 = np.zeros((128, 32), np.float32)
    G1 = np.zeros((64, 32), np.float32)
    for c in range(128):
        G0[c, c // GS] = 1.0
    for c in range(128, 192):
        G1[c - 128, c // GS] = 1.0
    E0 = G0.T.copy()
    E1 = G1.T.copy()
    sq = np.sqrt(BIG).astype(np.float32)
    bk = np.zeros((8, 128), np.float32)
    for r in range(8):
        bk[r, 16 * r:16 * r + 16] = sq
    bk4 = np.tile(bk, (1, 4))
    return G0, G1, E0, E1, bk, bk4


def _pack_qkv(w, b, scale):
    qs = [w[192 * h:192 * h + 64] for h in range(3)]
    ks = [w[192 * h + 64:192 * h + 128] for h in range(3)]
    qb = [b[192 * h:192 * h + 64] for h in range(3)]
    kb = [b[192 * h + 64:192 * h + 128] for h in range(3)]
    rows = np.concatenate([qs[0], qs[1], ks[0], ks[1], qs[2], ks[2]], 0) * scale
    brow = np.concatenate([qb[0], qb[1], kb[0], kb[1], qb[2], kb[2]], 0) * scale
    return np.concatenate([rows.T, brow[None, :]], 0).astype(np.float32)


def _pack_v(w, b):
    vs = [w[192 * h + 128:192 * h + 192] for h in range(3)]
    vb = [b[192 * h + 128:192 * h + 192] for h in range(3)]
    rows = np.concatenate(vs, 0)
    brow = np.concatenate(vb, 0)
    return np.concatenate([rows.T, brow[None, :]], 0).astype(np.float32)


def _pack_p(w, b):
    return np.concatenate([w.T, b[None, :]], 0).astype(np.float32)


def build_nc():
    nc = bacc.Bacc("TRN2", target_bir_lowering=False, debug=False,
                   num_devices=N_CORES)

    x_in = nc.dram_tensor("x_in", [NB1, C, HW], F32, kind="ExternalInput").ap()
    wqk1 = nc.dram_tensor("wqk1", [193, 384], F32, kind="ExternalInput").ap()
    wv1 = nc.dram_tensor("wv1", [193, 192], F32, kind="ExternalInput").ap()
    wp1 = nc.dram_tensor("wp1", [193, 192], F32, kind="ExternalInput").ap()
    gn1 = nc.dram_tensor("gn1", [C, 2], F32, kind="ExternalInput").ap()
    wqk2 = nc.dram_tensor("wqk2", [193, 384], F32, kind="ExternalInput").ap()
    wv2 = nc.dram_tensor("wv2", [193, 192], F32, kind="ExternalInput").ap()
    wp2 = nc.dram_tensor("wp2", [193, 192], F32, kind="ExternalInput").ap()
    gn2 = nc.dram_tensor("gn2", [C, 2], F32, kind="ExternalInput").ap()
    z_out = nc.dram_tensor("z_out", [B, C, PHW, T], F32,
                           kind="ExternalOutput").ap()

    G0n, G1n, E0n, E1n, bkn, bk4n = _consts()
    G0d = nc.inline_tensor(G0n, "G0c").ap()
    G1d = nc.inline_tensor(G1n, "G1c").ap()
    E0d = nc.inline_tensor(E0n, "E0c").ap()
    E1d = nc.inline_tensor(E1n, "E1c").ap()
    bkd = nc.inline_tensor(bkn, "bkc").ap()
    bk4d = nc.inline_tensor(bk4n, "bk4c").ap()
    magicd = nc.inline_tensor(np.full((32, 32), 0x5F3759DF, np.int32),
                              "magicc").ap()

    no_coll = os.environ.get("NO_COLLECTIVE", "0") == "1"

    with tile.TileContext(nc) as tc:
        with (
            tc.tile_pool(name="wts", bufs=1) as wts,
            tc.tile_pool(name="dram", bufs=1, space="DRAM") as dram,
            tc.tile_pool(name="xp", bufs=2) as xp,
            tc.tile_pool(name="np_", bufs=2) as npl,
            tc.tile_pool(name="gp", bufs=2) as gp,
            tc.tile_pool(name="qkp", bufs=2) as qkp,
            tc.tile_pool(name="vp", bufs=2) as vp,
            tc.tile_pool(name="ep", bufs=2) as ep,
            tc.tile_pool(name="ap_", bufs=2) as apl,
            tc.tile_pool(name="yp", bufs=2) as yp,
            tc.tile_pool(name="dp", bufs=2) as dp,
        ):
            def wtile(srcap, p, f, tag, rnd=False, dt=F32):
                t = wts.tile([p, f], dt, tag=tag, name=tag)
                if rnd:
                    nc.sync.dma_start(out=_r(t[:]), in_=_r(srcap))
                else:
                    nc.sync.dma_start(out=t[:], in_=srcap)
                return t

            wqk1a = wtile(wqk1[0:128, :], 128, 384, "wqk1a", rnd=True)
            wqk1b = wtile(wqk1[128:193, :], 65, 384, "wqk1b", rnd=True)
            wv1a = wtile(wv1[0:128, :], 128, 192, "wv1a")
            wv1b = wtile(wv1[128:193, :], 65, 192, "wv1b")
            wp1a = wtile(wp1[0:128, :], 128, 192, "wp1a", rnd=True)
            wp1b = wtile(wp1[128:193, :], 65, 192, "wp1b", rnd=True)
            wqk2a = wtile(wqk2[0:128, :], 128, 384, "wqk2a", rnd=True)
            wqk2b = wtile(wqk2[128:193, :], 65, 384, "wqk2b", rnd=True)
            wv2a = wtile(wv2[0:128, :], 128, 192, "wv2a")
            wv2b = wtile(wv2[128:193, :], 65, 192, "wv2b")
            wp2a = wtile(wp2[0:128, :], 128, 192, "wp2a", rnd=True)
            wp2b = wtile(wp2[128:193, :], 65, 192, "wp2b", rnd=True)
            g1a = wtile(gn1[0:128, :], 128, 2, "g1a")
            g1b = wtile(gn1[128:192, :], 64, 2, "g1b")
            g2a = wtile(gn2[0:128, :], 128, 2, "g2a")
            g2b = wtile(gn2[128:192, :], 64, 2, "g2b")
            G0s = wtile(G0d, 128, 32, "G0s")
            G1s = wtile(G1d, 64, 32, "G1s")
            E0s = wtile(E0d, 32, 128, "E0s")
            E1s = wtile(E1d, 32, 64, "E1s")
            bks = wtile(bkd, 8, 128, "bks", rnd=True)
            bk4s = wtile(bk4d, 8, 512, "bk4s", rnd=True)
            magi = wtile(magicd, 32, 32, "magi", dt=I32)
            negu = wts.tile([1, 128], F32, tag="negu", name="negu")
            nc.vector.memset(negu[:], -BIG)
            ones1 = wts.tile([1, 512], F32, tag="ones1", name="ones1")
            nc.vector.memset(ones1[:], 1.0)

            a2a_in = [dram.tile([N_CORES, C, PHW], F32, tag=f"a2ai{i}",
                                name=f"a2ai{i}") for i in range(NB1)]
            a2a_out = [dram.tile([N_CORES, C, PHW], F32, tag=f"a2ao{i}",
                                 name=f"a2ao{i}") for i in range(NB1)]

            def gn_scale_bias(ppmm, st_a, st_b, ga, gb, nb, inv_cnt, ptag, pbufs):
                pg = ppmm.tile([32, 2 * nb], F32, tag=ptag, bufs=pbufs,
                               name="pg")
                nc.tensor.matmul(pg[:], G0s[:], st_a[:], start=True,
                                 stop=False)
                nc.tensor.matmul(pg[:], G1s[:], st_b[:], start=False,
                                 stop=True)
                grp = gp.tile([32, 2 * nb], F32, tag="grp", name="grp")
                tmp = gp.tile([32, 2 * nb], F32, tag="gtmp", name="gtmp")
                mu = grp[:, 0:nb]
                nc.vector.tensor_scalar_mul(mu, pg[:, 0:nb], inv_cnt)
                nc.vector.tensor_scalar_mul(tmp[:, 0:nb], pg[:, nb:2 * nb],
                                            inv_cnt)
                nc.vector.tensor_mul(tmp[:, nb:2 * nb], mu, mu)
                nc.vector.tensor_sub(tmp[:, 0:nb], tmp[:, 0:nb],
                                     tmp[:, nb:2 * nb])
                # rstd = rsqrt(var + eps): bit-trick seed + 2 Newton steps
                xv = gp.tile([32, nb], F32, tag="rsx", name="rsx")
                yv = gp.tile([32, nb], F32, tag="rsy", name="rsy")
                tv = gp.tile([32, nb], F32, tag="rst", name="rst")
                nc.vector.tensor_scalar_add(xv[:], tmp[:, 0:nb], EPS)
                nc.vector.tensor_scalar(yv[:].bitcast(I32),
                                        xv[:].bitcast(I32), 1, None,
                                        op0=OP.arith_shift_right)
                nc.vector.tensor_sub(yv[:].bitcast(I32), magi[:, 0:nb],
                                     yv[:].bitcast(I32))
                for _ in range(2):
                    nc.vector.tensor_mul(tv[:], yv[:], yv[:])
                    nc.vector.tensor_mul(tv[:], tv[:], xv[:])
                    nc.vector.tensor_scalar(tv[:], tv[:], -0.5, 1.5,
                                            op0=OP.mult, op1=OP.add)
                    nc.vector.tensor_mul(yv[:], yv[:], tv[:])
                nc.vector.tensor_copy(grp[:, nb:2 * nb], yv[:])
                out = []
                for Es, gch, p in ((E0s, ga, 128), (E1s, gb, 64)):
                    psc = ppmm.tile([p, 2 * nb], F32, tag=ptag, bufs=pbufs,
                                    name="psc")
                    nc.tensor.matmul(psc[:], Es[:], grp[:], start=True,
                                     stop=True)
                    sc = gp.tile([p, 2 * nb], F32, tag=f"sc{p}",
                                 name=f"sc{p}")
                    nc.vector.tensor_scalar_mul(sc[:, 0:nb], psc[:, nb:2 * nb],
                                                gch[:, 0:1])
                    nc.vector.tensor_mul(sc[:, nb:2 * nb], psc[:, 0:nb],
                                         sc[:, 0:nb])
                    nc.vector.tensor_tensor(
                        sc[:, nb:2 * nb],
                        gch[:, 1:2].to_broadcast((p, nb)),
                        sc[:, nb:2 * nb], op=OP.subtract)
                    out.append(sc)
                return out

            s2s = {}
            s2 = {}

            def s2_staging_i(b, i):
                if b not in s2s:
                    x2Sa = xp.tile([128, T, PHW], F32, tag="x2Sa", bufs=2,
                                   name="x2Sa")
                    x2Sb = xp.tile([64, T, PHW], F32, tag="x2Sb", bufs=2,
                                   name="x2Sb")
                    s2s[b] = (x2Sa, x2Sb)
                x2Sa, x2Sb = s2s[b]
                srcv = a2a_out[i][:].rearrange("s c p -> c p s")
                for xt, c0, c1 in ((x2Sa, 0, 128), (x2Sb, 128, 192)):
                    for j in range(4):
                        nc.sync.dma_start(
                            out=xt[:, 4 * j + i, :],
                            in_=srcv[c0:c1, :, 4 * b + j])

            # ============================================== STAGE 1
            pp_s1 = tc.tile_pool(name="pp_s1", bufs=1, space="PSUM")
            pp1 = pp_s1.__enter__()

            s1 = {}
            s1n = {}
            s1st = {}

            def s1_load(ib):
                x0 = xp.tile([128, HW], F32, tag="x0", name="x0")
                x1 = xp.tile([64, HW], F32, tag="x1", name="x1")
                nc.sync.dma_start(out=x0[:], in_=x_in[ib, 0:128, :])
                nc.sync.dma_start(out=x1[:], in_=x_in[ib, 128:192, :])

                sts = []
                for xt, p, tg in ((x0, 128, "a"), (x1, 64, "b")):
                    stat = gp.tile([p, 2, 6], F32, tag=f"bst{tg}",
                                   name=f"bst{tg}")
                    nc.vector.bn_stats(stat[:, 0, :], xt[:, 0:512])
                    nc.vector.bn_stats(stat[:, 1, :], xt[:, 512:1024])
                    mv = gp.tile([p, 2], F32, tag=f"mv{tg}", name=f"mv{tg}")
                    nc.vector.bn_aggr(mv[:], stat[:])
                    st = gp.tile([p, 2], F32, tag=f"st{tg}", name=f"st{tg}")
                    nc.vector.tensor_copy(st[:, 0:1], mv[:, 0:1])
                    nc.vector.tensor_mul(st[:, 1:2], mv[:, 0:1], mv[:, 0:1])
                    nc.vector.tensor_add(st[:, 1:2], st[:, 1:2], mv[:, 1:2])
                    sts.append(st)
                s1st[ib] = (x0, x1, sts)

            def s1_gn(ib):
                x0, x1, sts = s1st.pop(ib)
                sc0, sc1 = gn_scale_bias(pp1, sts[0], sts[1], g1a, g1b, 1,
                                         1.0 / GS, "pqk", 2)

                xn0 = npl.tile([128, HW], F32, tag="xn0", name="xn0")
                xn1 = npl.tile([65, HW], F32, tag="xn1", name="xn1")
                nc.vector.tensor_scalar(_r(xn0[:]), x0[:], sc0[:, 0:1],
                                        sc0[:, 1:2], op0=OP.mult, op1=OP.add)
                nc.vector.tensor_scalar(_r(xn1[0:64, :]), x1[:], sc1[:, 0:1],
                                        sc1[:, 1:2], op0=OP.mult, op1=OP.add)
                nc.gpsimd.memset(xn1[64:65, :], 1.0)
                s1n[ib] = (x0, x1, xn0, xn1)

            def s1_preB(ib):
                x0, x1, xn0, xn1 = s1n.pop(ib)
                qk_tiles = {}
                for name, cb, mw in (("qab", 0, 128), ("kab", 128, 128),
                                     ("qc", 256, 64), ("kc", 320, 64)):
                    dst = qkp.tile([mw, HW], F32, tag=name, name=name)
                    for n in range(2):
                        pq = pp1.tile([mw, 512], F32, tag="pqk", bufs=2,
                                      name="pq")
                        nc.tensor.matmul(pq[:], _r(wqk1a[:, cb:cb + mw]),
                                         _r(xn0[:, 512 * n:512 * n + 512]),
                                         start=True, stop=False)
                        nc.tensor.matmul(pq[:], _r(wqk1b[:, cb:cb + mw]),
                                         _r(xn1[:, 512 * n:512 * n + 512]),
                                         start=False, stop=True)
                        nc.scalar.copy(
                            _r(dst[:, 512 * n:512 * n + 512]), pq[:])
                    qk_tiles[name] = dst

                v_sb = vp.tile([128, 8, 3, 65], F32, tag="v_sb", name="v_sb")
                nc.gpsimd.memset(v_sb[:, :, :, 64:65], 1.0)
                for lc in range(8):
                    pv = pp1.tile([128, 192], F32, tag="pqk", bufs=2,
                                  name="pv")
                    nc.tensor.matmul(pv[:], xn0[:, 128 * lc:128 * lc + 128],
                                     wv1a[:], start=True, stop=False)
                    nc.tensor.matmul(pv[:], xn1[:, 128 * lc:128 * lc + 128],
                                     wv1b[:], start=False, stop=True)
                    nc.vector.tensor_copy(
                        _r(v_sb[:, lc, :, 0:64]),
                        pv[:].rearrange("p (h c) -> p h c", h=3))
                s1[ib] = (x0, x1, qk_tiles, v_sb)

            s1a = {}

            def s1_heads(ib, mid=None):
                x0, x1, qk_tiles, v_sb = s1.pop(ib)
                a0 = apl.tile([128, HW], F32, tag="a0", name="a0")
                a1 = apl.tile([65, HW], F32, tag="a1", name="a1")
                nc.gpsimd.memset(a1[64:65, :], 1.0)
                for h in range(HEADS):
                    if h == 2 and mid is not None:
                        mid()
                    if h == 0:
                        q = qk_tiles["qab"][0:64, :]
                        k = qk_tiles["kab"][0:64, :]
                        tp = (0, 0)
                    elif h == 1:
                        q = qk_tiles["qab"][64:128, :]
                        k = qk_tiles["kab"][64:128, :]
                        tp = (64, 0)
                    else:
                        q = qk_tiles["qc"][:]
                        k = qk_tiles["kc"][:]
                        tp = (0, 0)
                    pav = pp1.tile([65, HW], F32, tag="pav", bufs=2,
                                   name="pav")
                    for sc_i in range(8):
                        pqk = pp1.tile([128, HW], F32, tag="pqk", bufs=2,
                                       name="pqk")
                        for n in range(2):
                            nc.tensor.matmul(
                                pqk[:, 512 * n:512 * n + 512],
                                _r(k[:, 128 * sc_i:128 * sc_i + 128]),
                                _r(q[:, 512 * n:512 * n + 512]),
                                start=True, stop=True, tile_position=tp)
                        eT = ep.tile([128, HW], F32, tag="eT", name="eT")
                        nc.scalar.activation(_r(eT[:]), pqk[:], AF.Exp)
                        for n in range(2):
                            nc.tensor.matmul(
                                pav[:, 512 * n:512 * n + 512],
                                _r(v_sb[:, sc_i, h, :]),
                                _r(eT[:, 512 * n:512 * n + 512]),
                                start=(sc_i == 0), stop=(sc_i == 7))
                    dnc = dp.tile([1, HW], F32, tag="dnc", name="dnc")
                    nc.vector.tensor_copy(dnc[:], pav[64:65, :])
                    dnr = dp.tile([1, HW], F32, tag="dnr", name="dnr")
                    nc.vector.reciprocal_approx_fast(dnr[:], dnc[:])
                    rbs = dp.tile([64, HW], F32, tag="rbs", name="rbs")
                    nc.gpsimd.partition_broadcast(rbs[:], dnr[:], channels=64)
                    a_sl = (a0[64 * h:64 * h + 64, :] if h < 2
                            else a1[0:64, :])
                    nc.vector.tensor_mul(_r(a_sl), pav[0:64, :], rbs[:])
                s1a[ib] = (x0, x1, a0, a1)

            def s1_tail(ib):
                x0, x1, a0, a1 = s1a.pop(ib)
                y0 = yp.tile([128, HW], F32, tag="y0", name="y0")
                y1 = yp.tile([64, HW], F32, tag="y1", name="y1")
                for (mb, mw, yt, xt) in ((0, 128, y0, x0), (128, 64, y1, x1)):
                    for n in range(2):
                        pp = pp1.tile([mw, 512], F32, tag="pqk", bufs=2,
                                      name="pp")
                        nc.tensor.matmul(pp[:], _r(wp1a[:, mb:mb + mw]),
                                         _r(a0[:, 512 * n:512 * n + 512]),
                                         start=True, stop=False)
                        nc.tensor.matmul(pp[:], _r(wp1b[:, mb:mb + mw]),
                                         _r(a1[:, 512 * n:512 * n + 512]),
                                         start=False, stop=True)
                        nc.vector.tensor_add(yt[:, 512 * n:512 * n + 512],
                                             pp[:],
                                             xt[:, 512 * n:512 * n + 512])
                dst = a2a_in[ib][:].rearrange("j c p -> c j p")
                nc.sync.dma_start(out=dst[0:128, :, :],
                                  in_=y0[:].rearrange("c (j p) -> c j p", j=8))
                nc.sync.dma_start(out=dst[128:192, :, :],
                                  in_=y1[:].rearrange("c (j p) -> c j p", j=8))
                if no_coll:
                    nc.sync.dma_start(out=a2a_out[ib][:], in_=a2a_in[ib][:])
                else:
                    nc.gpsimd.collective_compute(
                        "AllToAll", OP.bypass,
                        replica_groups=[list(range(N_CORES))],
                        ins=[a2a_in[ib][:].opt()],
                        outs=[a2a_out[ib][:].opt()],
                    )
                for bb_ in range(B):
                    s2_staging_i(bb_, ib)

            s1_load(0)
            s1_gn(0)
            s1_preB(0)
            for ib in range(NB1):
                if ib + 1 < NB1:
                    s1_load(ib + 1)
                    s1_gn(ib + 1)
                s1_heads(ib)
                if ib + 1 < NB1:
                    s1_preB(ib + 1)
                s1_tail(ib)
            pp_s1.__exit__(None, None, None)

            # ============================================== STAGE 2
            pp_s2 = tc.tile_pool(name="pp_s2", bufs=1, space="PSUM")
            pp2 = pp_s2.__enter__()


            s2n = {}

            def s2_preA(b, blk):
                x2Sa, x2Sb = s2s[b]
                po = 32 * blk
                x2a = x2Sa[:, :, po:po + 32].rearrange("c t l -> c l t")
                x2b = x2Sb[:, :, po:po + 32].rearrange("c t l -> c l t")

                sts2 = []
                for xt, p, tg in ((x2a, 128, "a"), (x2b, 64, "b")):
                    st = gp.tile([p, 64], F32, tag=f"st2{tg}", name=f"st2{tg}")
                    nc.vector.reduce_sum(st[:, 0:32], xt, axis=AX.X)
                    scr = npl.tile([p, 32, T], F32, tag=f"sq2{tg}", bufs=1,
                                   name=f"sq2{tg}")
                    nc.gpsimd.tensor_mul(scr[:], xt, xt)
                    nc.vector.reduce_sum(st[:, 32:64], scr[:], axis=AX.X)
                    sts2.append(st)
                sc2a, sc2b = gn_scale_bias(pp2, sts2[0], sts2[1], g2a, g2b,
                                           32, 1.0 / (GS * T), "mm2", 3)

                xn2a = npl.tile([128, 32 * T], F32, tag="xn0", name="xn2a")
                xn2b = npl.tile([65, 32 * T], F32, tag="xn1", name="xn2b")
                for xt, xnt, sct, p in ((x2a, xn2a, sc2a, 128),
                                        (x2b, xn2b, sc2b, 64)):
                    xn3 = xnt[0:p, :].rearrange("c (l t) -> c l t", t=T)
                    nc.vector.tensor_tensor(
                        _r(xn3), xt,
                        sct[:, 0:32, None].to_broadcast((p, 32, T)),
                        op=OP.mult)
                    nc.vector.tensor_tensor(
                        _r(xn3), xn3,
                        sct[:, 32:64, None].to_broadcast((p, 32, T)),
                        op=OP.add)
                nc.gpsimd.memset(xn2b[64:65, :], 1.0)
                s2n[(b, blk)] = (x2a, x2b, xn2a, xn2b)

            def s2_preB(b, blk):
                x2a, x2b, xn2a, xn2b = s2n.pop((b, blk))
                qk2 = {}
                for name, cb, mw in (("qab", 0, 128), ("kab", 128, 128),
                                     ("qc", 256, 64), ("kc", 320, 64)):
                    dst = qkp.tile([mw, 512], F32, tag=name, name=f"q2{name}")
                    pq = pp2.tile([mw, 512], F32, tag="mm2", bufs=3,
                                  name="pq2")
                    nc.tensor.matmul(pq[:], _r(wqk2a[:, cb:cb + mw]),
                                     _r(xn2a[:]), start=True, stop=False)
                    nc.tensor.matmul(pq[:], _r(wqk2b[:, cb:cb + mw]),
                                     _r(xn2b[:]), start=False, stop=True)
                    nc.scalar.copy(dst[:], pq[:])
                    qk2[name] = dst

                v2 = vp.tile([128, 4, 3, 65], F32, tag="v_sb", name="v2")
                nc.gpsimd.memset(v2[:, :, :, 64:65], 1.0)
                for o in range(4):
                    pv = pp2.tile([128, 192], F32, tag="mm2", bufs=3,
                                  name="pv2")
                    nc.tensor.matmul(pv[:], xn2a[:, 128 * o:128 * o + 128],
                                     wv2a[:], start=True, stop=False)
                    nc.tensor.matmul(pv[:], xn2b[:, 128 * o:128 * o + 128],
                                     wv2b[:], start=False, stop=True)
                    nc.scalar.copy(
                        v2[:, o, :, 0:64],
                        pv[:].rearrange("p (h c) -> p h c", h=3))
                s2[(b, blk)] = (x2a, x2b, qk2, v2)

            s2a = {}

            def s2_heads(b, blk):
                x2a, x2b, qk2, v2 = s2.pop((b, blk))
                a2_0 = apl.tile([128, 512], F32, tag="a0", name="a2_0")
                a2_1 = apl.tile([65, 512], F32, tag="a1", name="a2_1")
                nc.gpsimd.memset(a2_1[64:65, :], 1.0)
                for h in range(HEADS):
                    if h == 0:
                        q = qk2["qab"][0:64, :]
                        k = qk2["kab"][0:64, :]
                        tp = (0, 0)
                    elif h == 1:
                        q = qk2["qab"][64:128, :]
                        k = qk2["kab"][64:128, :]
                        tp = (64, 0)
                    else:
                        q = qk2["qc"][:]
                        k = qk2["kc"][:]
                        tp = (0, 0)
                    ps2 = pp2.tile([128, 512], F32, tag="ps2", bufs=2,
                                   name="ps2")
                    nc.tensor.matmul(ps2[:], _r(negu[:]), _r(ones1[:]),
                                     start=True, stop=False)
                    nc.tensor.matmul(ps2[:], _r(bks[:]), _r(bk4s[:]),
                                     start=False, stop=False)
                    for o in range(4):
                        nc.tensor.matmul(ps2[:, 128 * o:128 * o + 128],
                                         k[:, 128 * o:128 * o + 128],
                                         q[:, 128 * o:128 * o + 128],
                                         start=False, stop=(o == 3),
                                         tile_position=tp)
                    eT2 = ep.tile([128, 512], F32, tag="eT", name="eT2")
                    nc.scalar.activation(eT2[:], ps2[:], AF.Exp)
                    pav2 = pp2.tile([65, 512], F32, tag="pav2", bufs=2,
                                    name="pav2")
                    for o in range(4):
                        nc.tensor.matmul(pav2[:, 128 * o:128 * o + 128],
                                         v2[:, o, h, :],
                                         eT2[:, 128 * o:128 * o + 128],
                                         start=(o == 0), stop=(o == 3))
                    dn2c = dp.tile([1, 512], F32, tag="dnc", name="dn2c")
                    nc.scalar.copy(dn2c[:], pav2[64:65, :])
                    dn2r = dp.tile([1, 512], F32, tag="dnr", name="dn2r")
                    nc.vector.reciprocal_approx_fast(dn2r[:], dn2c[:])
                    rbs2 = dp.tile([64, 512], F32, tag="rbs", name="rbs2")
                    nc.gpsimd.partition_broadcast(rbs2[:], dn2r[:],
                                                  channels=64)
                    a_sl = (a2_0[64 * h:64 * h + 64, :] if h < 2
                            else a2_1[0:64, :])
                    nc.vector.tensor_mul(_r(a_sl), pav2[0:64, :], rbs2[:])
                s2a[(b, blk)] = (x2a, x2b, a2_0, a2_1)

            def s2_tail(b, blk):
                po = 32 * blk
                x2a, x2b, a2_0, a2_1 = s2a.pop((b, blk))
                for (mb, mw, xt, c0, c1) in ((0, 128, x2a, 0, 128),
                                             (128, 64, x2b, 128, 192)):
                    pp = pp2.tile([mw, 512], F32, tag="mm2", bufs=3,
                                  name="pp2")
                    nc.tensor.matmul(pp[:], _r(wp2a[:, mb:mb + mw]),
                                     _r(a2_0[:]), start=True, stop=False)
                    nc.tensor.matmul(pp[:], _r(wp2b[:, mb:mb + mw]),
                                     _r(a2_1[:]), start=False, stop=True)
                    y2 = yp.tile([mw, 32, T], F32,
                                 tag=("y0" if mb == 0 else "y1"),
                                 name="y2")
                    nc.vector.tensor_add(
                        y2[:], pp[:].rearrange("c (l t) -> c l t", t=T), xt)
                    nc.sync.dma_start(out=z_out[b, c0:c1, po:po + 32, :],
                                      in_=y2[:])

            blocks = [(b, blk) for b in range(B) for blk in range(4)]
            s2_preA(*blocks[0])
            s2_preB(*blocks[0])
            for i, bb in enumerate(blocks):
                if i + 1 < len(blocks):
                    s2_preA(*blocks[i + 1])
                s2_heads(*bb)
                if i + 1 < len(blocks):
                    s2_preB(*blocks[i + 1])
                s2_tail(*bb)
            pp_s2.__exit__(None, None, None)

    nc.compile()
    return nc


_NC_LOCK = threading.Lock()
_NC_CACHE = {}


def _get_nc():
    with _NC_LOCK:
        if "nc" not in _NC_CACHE:
            _NC_CACHE["nc"] = build_nc()
        return _NC_CACHE["nc"]


def kernel(x, norm_g, norm_b, qkv_w, qkv_b, proj_w, proj_b,
           normt_g, normt_b, qkvt_w, qkvt_b, projt_w, projt_b):
    x = np.asarray(x, np.float32)
    nc = _get_nc()

    xs = np.ascontiguousarray(
        np.transpose(x, (0, 2, 1, 3, 4)).reshape(B * T, C, HW))
    shared = {
        "wqk1": _pack_qkv(np.asarray(qkv_w, np.float32),
                          np.asarray(qkv_b, np.float32), QKSCALE),
        "wv1": _pack_v(np.asarray(qkv_w, np.float32),
                       np.asarray(qkv_b, np.float32)),
        "wp1": _pack_p(np.asarray(proj_w, np.float32),
                       np.asarray(proj_b, np.float32)),
        "gn1": np.stack([np.asarray(norm_g, np.float32),
                         np.asarray(norm_b, np.float32)], 1),
        "wqk2": _pack_qkv(np.asarray(qkvt_w, np.float32),
                          np.asarray(qkvt_b, np.float32), QKSCALE),
        "wv2": _pack_v(np.asarray(qkvt_w, np.float32),
                       np.asarray(qkvt_b, np.float32)),
        "wp2": _pack_p(np.asarray(projt_w, np.float32),
                       np.asarray(projt_b, np.float32)),
        "gn2": np.stack([np.asarray(normt_g, np.float32),
                         np.asarray(normt_b, np.float32)], 1),
    }
    in_maps = []
    for core in range(N_CORES):
        m = dict(shared)
        m["x_in"] = np.ascontiguousarray(xs[NB1 * core:NB1 * core + NB1])
        in_maps.append(m)

    res = run_bass_kernel_spmd(nc, in_maps, core_ids=list(range(N_CORES)))

    # z_out per core: [B, C, 128, T], hw = 128*core + p
    zhw = np.concatenate([res.results[c]["z_out"] for c in range(N_CORES)],
                         axis=2)                      # [B, C, 1024, T]
    z = np.transpose(zhw, (0, 1, 3, 2)).reshape(B, C, T, 32, 32)
    return np.ascontiguousarray(z)


# revision 49
# speedup vs baseline: 1.0105x; 1.0105x over previous
"""Fused spatial+temporal AttentionBlock kernel for 8 Trainium2 NeuronCores.

Sharding: stage 1 (spatial attention, batch (b t) = 32) data-parallel, 4
batches/core; stage 2 (temporal attention, batch (b h w) = 2048)
data-parallel over hw, 128 positions/core; 4 AllToAll collectives between
stages (first three hidden under stage-1 compute).

Attention uses the scoresT [s, t] orientation: softmax denominator comes from
a ones-column in the AV matmul; exp skips max-subtraction (scores bounded,
fp32-safe, softmax shift-invariant).  Large matmuls run as float32r (1
cycle/row vs 4 for fp32).  GroupNorm rstd is computed on the vector engine
(bit-trick + 2 Newton steps) so ACT only ever needs the exp table set.
Both stage loops are software-pipelined (prefix of iteration k+1 emitted
before body of k) since engines execute their streams in order.
"""

import os
import sys
import threading

sys.path.insert(0, '/opt/trn_rl_repo')
sys.path.insert(0, '/opt/trn_rl_repo/concourse')

import numpy as np

import concourse.bass as bass
import concourse.tile as tile
from concourse import bacc, mybir
from concourse.bass_utils import run_bass_kernel_spmd

F32 = mybir.dt.float32
F32R = mybir.dt.float32r
I32 = mybir.dt.int32
AF = mybir.ActivationFunctionType
OP = mybir.AluOpType
AX = mybir.AxisListType


def _r(ap):
    return ap.bitcast(F32R)


N_CORES = 8
C = 192
B = 2
T = 16
HW = 1024
NB1 = 4
PHW = 128
HEADS = 3
CH = 64
NGRP = 32
GS = 6
EPS = 1e-5
QKSCALE = CH ** -0.25
BIG = 60.0


def _consts():
    G0 = np.zeros((128, 32), np.float32)
    G1 = np.zeros((64, 32), np.float32)
    for c in range(128):
        G0[c, c // GS] = 1.0
    for c in range(128, 192):
        G1[c - 128, c // GS] = 1.0
    E0 = G0.T.copy()
    E1 = G1.T.copy()
    sq = np.sqrt(BIG).astype(np.float32)
    bk = np.zeros((8, 128), np.float32)
    for r in range(8):
        bk[r, 16 * r:16 * r + 16] = sq
    bk4 = np.tile(bk, (1, 4))
    return G0, G1, E0, E1, bk, bk4


def _pack_qkv(w, b, scale):
    qs = [w[192 * h:192 * h + 64] for h in range(3)]
    ks = [w[192 * h + 64:192 * h + 128] for h in range(3)]
    qb = [b[192 * h:192 * h + 64] for h in range(3)]
    kb = [b[192 * h + 64:192 * h + 128] for h in range(3)]
    rows = np.concatenate([qs[0], qs[1], ks[0], ks[1], qs[2], ks[2]], 0) * scale
    brow = np.concatenate([qb[0], qb[1], kb[0], kb[1], qb[2], kb[2]], 0) * scale
    return np.concatenate([rows.T, brow[None, :]], 0).astype(np.float32)


def _pack_v(w, b):
    vs = [w[192 * h + 128:192 * h + 192] for h in range(3)]
    vb = [b[192 * h + 128:192 * h + 192] for h in range(3)]
    rows = np.concatenate(vs, 0)
    brow = np.concatenate(vb, 0)
    return np.concatenate([rows.T, brow[None, :]], 0).astype(np.float32)


def _pack_p(w, b):
    return np.concatenate([w.T, b[None, :]], 0).astype(np.float32)


def build_nc():
    nc = bacc.Bacc("TRN2", target_bir_lowering=False, debug=False,
                   num_devices=N_CORES)

    x_in = nc.dram_tensor("x_in", [NB1, C, HW], F32, kind="ExternalInput").ap()
    wqk1 = nc.dram_tensor("wqk1", [193, 384], F32, kind="ExternalInput").ap()
    wv1 = nc.dram_tensor("wv1", [193, 192], F32, kind="ExternalInput").ap()
    wp1 = nc.dram_tensor("wp1", [193, 192], F32, kind="ExternalInput").ap()
    gn1 = nc.dram_tensor("gn1", [C, 2], F32, kind="ExternalInput").ap()
    wqk2 = nc.dram_tensor("wqk2", [193, 384], F32, kind="ExternalInput").ap()
    wv2 = nc.dram_tensor("wv2", [193, 192], F32, kind="ExternalInput").ap()
    wp2 = nc.dram_tensor("wp2", [193, 192], F32, kind="ExternalInput").ap()
    gn2 = nc.dram_tensor("gn2", [C, 2], F32, kind="ExternalInput").ap()
    z_out = nc.dram_tensor("z_out", [B, C, PHW, T], F32,
                           kind="ExternalOutput").ap()

    G0n, G1n, E0n, E1n, bkn, bk4n = _consts()
    G0d = nc.inline_tensor(G0n, "G0c").ap()
    G1d = nc.inline_tensor(G1n, "G1c").ap()
    E0d = nc.inline_tensor(E0n, "E0c").ap()
    E1d = nc.inline_tensor(E1n, "E1c").ap()
    bkd = nc.inline_tensor(bkn, "bkc").ap()
    bk4d = nc.inline_tensor(bk4n, "bk4c").ap()
    magicd = nc.inline_tensor(np.full((32, 32), 0x5F3759DF, np.int32),
                              "magicc").ap()

    no_coll = os.environ.get("NO_COLLECTIVE", "0") == "1"

    with tile.TileContext(nc) as tc:
        with (
            tc.tile_pool(name="wts", bufs=1) as wts,
            tc.tile_pool(name="dram", bufs=1, space="DRAM") as dram,
            tc.tile_pool(name="xp", bufs=2) as xp,
            tc.tile_pool(name="np_", bufs=2) as npl,
            tc.tile_pool(name="gp", bufs=2) as gp,
            tc.tile_pool(name="qkp", bufs=2) as qkp,
            tc.tile_pool(name="vp", bufs=2) as vp,
            tc.tile_pool(name="ep", bufs=2) as ep,
            tc.tile_pool(name="ap_", bufs=2) as apl,
            tc.tile_pool(name="yp", bufs=2) as yp,
            tc.tile_pool(name="dp", bufs=2) as dp,
        ):
            def wtile(srcap, p, f, tag, rnd=False, dt=F32):
                t = wts.tile([p, f], dt, tag=tag, name=tag)
                if rnd:
                    nc.sync.dma_start(out=_r(t[:]), in_=_r(srcap))
                else:
                    nc.sync.dma_start(out=t[:], in_=srcap)
                return t

            wqk1a = wtile(wqk1[0:128, :], 128, 384, "wqk1a", rnd=True)
            wqk1b = wtile(wqk1[128:193, :], 65, 384, "wqk1b", rnd=True)
            wv1a = wtile(wv1[0:128, :], 128, 192, "wv1a")
            wv1b = wtile(wv1[128:193, :], 65, 192, "wv1b")
            wp1a = wtile(wp1[0:128, :], 128, 192, "wp1a", rnd=True)
            wp1b = wtile(wp1[128:193, :], 65, 192, "wp1b", rnd=True)
            wqk2a = wtile(wqk2[0:128, :], 128, 384, "wqk2a", rnd=True)
            wqk2b = wtile(wqk2[128:193, :], 65, 384, "wqk2b", rnd=True)
            wv2a = wtile(wv2[0:128, :], 128, 192, "wv2a")
            wv2b = wtile(wv2[128:193, :], 65, 192, "wv2b")
            wp2a = wtile(wp2[0:128, :], 128, 192, "wp2a", rnd=True)
            wp2b = wtile(wp2[128:193, :], 65, 192, "wp2b", rnd=True)
            g1a = wtile(gn1[0:128, :], 128, 2, "g1a")
            g1b = wtile(gn1[128:192, :], 64, 2, "g1b")
            g2a = wtile(gn2[0:128, :], 128, 2, "g2a")
            g2b = wtile(gn2[128:192, :], 64, 2, "g2b")
            G0s = wtile(G0d, 128, 32, "G0s")
            G1s = wtile(G1d, 64, 32, "G1s")
            E0s = wtile(E0d, 32, 128, "E0s")
            E1s = wtile(E1d, 32, 64, "E1s")
            bks = wtile(bkd, 8, 128, "bks", rnd=True)
            bk4s = wtile(bk4d, 8, 512, "bk4s", rnd=True)
            magi = wtile(magicd, 32, 32, "magi", dt=I32)
            negu = wts.tile([1, 128], F32, tag="negu", name="negu")
            nc.vector.memset(negu[:], -BIG)
            ones1 = wts.tile([1, 512], F32, tag="ones1", name="ones1")
            nc.vector.memset(ones1[:], 1.0)

            a2a_in = [dram.tile([N_CORES, C, PHW], F32, tag=f"a2ai{i}",
                                name=f"a2ai{i}") for i in range(NB1)]
            a2a_out = [dram.tile([N_CORES, C, PHW], F32, tag=f"a2ao{i}",
                                 name=f"a2ao{i}") for i in range(NB1)]

            def gn_scale_bias(ppmm, st_a, st_b, ga, gb, nb, inv_cnt, ptag, pbufs):
                pg = ppmm.tile([32, 2 * nb], F32, tag=ptag, bufs=pbufs,
                               name="pg")
                nc.tensor.matmul(pg[:], G0s[:], st_a[:], start=True,
                                 stop=False)
                nc.tensor.matmul(pg[:], G1s[:], st_b[:], start=False,
                                 stop=True)
                grp = gp.tile([32, 2 * nb], F32, tag="grp", name="grp")
                tmp = gp.tile([32, 2 * nb], F32, tag="gtmp", name="gtmp")
                mu = grp[:, 0:nb]
                nc.vector.tensor_scalar_mul(mu, pg[:, 0:nb], inv_cnt)
                nc.vector.tensor_scalar_mul(tmp[:, 0:nb], pg[:, nb:2 * nb],
                                            inv_cnt)
                nc.vector.tensor_mul(tmp[:, nb:2 * nb], mu, mu)
                nc.vector.tensor_sub(tmp[:, 0:nb], tmp[:, 0:nb],
                                     tmp[:, nb:2 * nb])
                # rstd = rsqrt(var + eps): bit-trick seed + 2 Newton steps
                xv = gp.tile([32, nb], F32, tag="rsx", name="rsx")
                yv = gp.tile([32, nb], F32, tag="rsy", name="rsy")
                tv = gp.tile([32, nb], F32, tag="rst", name="rst")
                nc.vector.tensor_scalar_add(xv[:], tmp[:, 0:nb], EPS)
                nc.vector.tensor_scalar(yv[:].bitcast(I32),
                                        xv[:].bitcast(I32), 1, None,
                                        op0=OP.arith_shift_right)
                nc.vector.tensor_sub(yv[:].bitcast(I32), magi[:, 0:nb],
                                     yv[:].bitcast(I32))
                for _ in range(2):
                    nc.vector.tensor_mul(tv[:], yv[:], yv[:])
                    nc.vector.tensor_mul(tv[:], tv[:], xv[:])
                    nc.vector.tensor_scalar(tv[:], tv[:], -0.5, 1.5,
                                            op0=OP.mult, op1=OP.add)
                    nc.vector.tensor_mul(yv[:], yv[:], tv[:])
                nc.vector.tensor_copy(grp[:, nb:2 * nb], yv[:])
                out = []
                for Es, gch, p in ((E0s, ga, 128), (E1s, gb, 64)):
                    psc = ppmm.tile([p, 2 * nb], F32, tag=ptag, bufs=pbufs,
                                    name="psc")
                    nc.tensor.matmul(psc[:], Es[:], grp[:], start=True,
                                     stop=True)
                    sc = gp.tile([p, 2 * nb], F32, tag=f"sc{p}",
                                 name=f"sc{p}")
                    nc.vector.tensor_scalar_mul(sc[:, 0:nb], psc[:, nb:2 * nb],
                                                gch[:, 0:1])
                    nc.vector.tensor_mul(sc[:, nb:2 * nb], psc[:, 0:nb],
                                         sc[:, 0:nb])
                    nc.vector.tensor_tensor(
                        sc[:, nb:2 * nb],
                        gch[:, 1:2].to_broadcast((p, nb)),
                        sc[:, nb:2 * nb], op=OP.subtract)
                    out.append(sc)
                return out

            s2s = {}
            s2 = {}

            def s2_staging_i(b, i):
                if b not in s2s:
                    x2Sa = xp.tile([128, T, PHW], F32, tag="x2Sa", bufs=2,
                                   name="x2Sa")
                    x2Sb = xp.tile([64, T, PHW], F32, tag="x2Sb", bufs=2,
                                   name="x2Sb")
                    s2s[b] = (x2Sa, x2Sb)
                x2Sa, x2Sb = s2s[b]
                srcv = a2a_out[i][:].rearrange("s c p -> c p s")
                for xt, c0, c1 in ((x2Sa, 0, 128), (x2Sb, 128, 192)):
                    for j in range(4):
                        nc.sync.dma_start(
                            out=xt[:, 4 * j + i, :],
                            in_=srcv[c0:c1, :, 4 * b + j])

            # ============================================== STAGE 1
            pp_s1 = tc.tile_pool(name="pp_s1", bufs=1, space="PSUM")
            pp1 = pp_s1.__enter__()

            s1 = {}
            s1n = {}
            s1st = {}

            def s1_load(ib):
                x0 = xp.tile([128, HW], F32, tag="x0", name="x0")
                x1 = xp.tile([64, HW], F32, tag="x1", name="x1")
                nc.sync.dma_start(out=x0[:], in_=x_in[ib, 0:128, :])
                nc.sync.dma_start(out=x1[:], in_=x_in[ib, 128:192, :])

                sts = []
                for xt, p, tg in ((x0, 128, "a"), (x1, 64, "b")):
                    stat = gp.tile([p, 2, 6], F32, tag=f"bst{tg}",
                                   name=f"bst{tg}")
                    nc.vector.bn_stats(stat[:, 0, :], xt[:, 0:512])
                    nc.vector.bn_stats(stat[:, 1, :], xt[:, 512:1024])
                    mv = gp.tile([p, 2], F32, tag=f"mv{tg}", name=f"mv{tg}")
                    nc.vector.bn_aggr(mv[:], stat[:])
                    st = gp.tile([p, 2], F32, tag=f"st{tg}", name=f"st{tg}")
                    nc.vector.tensor_copy(st[:, 0:1], mv[:, 0:1])
                    nc.vector.tensor_mul(st[:, 1:2], mv[:, 0:1], mv[:, 0:1])
                    nc.vector.tensor_add(st[:, 1:2], st[:, 1:2], mv[:, 1:2])
                    sts.append(st)
                s1st[ib] = (x0, x1, sts)

            def s1_gn(ib):
                x0, x1, sts = s1st.pop(ib)
                sc0, sc1 = gn_scale_bias(pp1, sts[0], sts[1], g1a, g1b, 1,
                                         1.0 / GS, "pqk", 2)

                xn0 = npl.tile([128, HW], F32, tag="xn0", name="xn0")
                xn1 = npl.tile([65, HW], F32, tag="xn1", name="xn1")
                nc.vector.tensor_scalar(_r(xn0[:]), x0[:], sc0[:, 0:1],
                                        sc0[:, 1:2], op0=OP.mult, op1=OP.add)
                nc.vector.tensor_scalar(_r(xn1[0:64, :]), x1[:], sc1[:, 0:1],
                                        sc1[:, 1:2], op0=OP.mult, op1=OP.add)
                nc.gpsimd.memset(xn1[64:65, :], 1.0)
                s1n[ib] = (x0, x1, xn0, xn1)

            def s1_preB(ib):
                x0, x1, xn0, xn1 = s1n.pop(ib)
                qk_tiles = {}
                for name, cb, mw in (("qab", 0, 128), ("kab", 128, 128),
                                     ("qc", 256, 64), ("kc", 320, 64)):
                    dst = qkp.tile([mw, HW], F32, tag=name, name=name)
                    for n in range(2):
                        pq = pp1.tile([mw, 512], F32, tag="pqk", bufs=2,
                                      name="pq")
                        nc.tensor.matmul(pq[:], _r(wqk1a[:, cb:cb + mw]),
                                         _r(xn0[:, 512 * n:512 * n + 512]),
                                         start=True, stop=False)
                        nc.tensor.matmul(pq[:], _r(wqk1b[:, cb:cb + mw]),
                                         _r(xn1[:, 512 * n:512 * n + 512]),
                                         start=False, stop=True)
                        nc.scalar.copy(
                            _r(dst[:, 512 * n:512 * n + 512]), pq[:])
                    qk_tiles[name] = dst

                v_sb = vp.tile([128, 8, 3, 65], F32, tag="v_sb", name="v_sb")
                nc.gpsimd.memset(v_sb[:, :, :, 64:65], 1.0)
                for lc in range(8):
                    pv = pp1.tile([128, 192], F32, tag="pqk", bufs=2,
                                  name="pv")
                    nc.tensor.matmul(pv[:], xn0[:, 128 * lc:128 * lc + 128],
                                     wv1a[:], start=True, stop=False)
                    nc.tensor.matmul(pv[:], xn1[:, 128 * lc:128 * lc + 128],
                                     wv1b[:], start=False, stop=True)
                    nc.vector.tensor_copy(
                        _r(v_sb[:, lc, :, 0:64]),
                        pv[:].rearrange("p (h c) -> p h c", h=3))
                s1[ib] = (x0, x1, qk_tiles, v_sb)

            s1a = {}

            def s1_heads(ib, mid=None):
                x0, x1, qk_tiles, v_sb = s1.pop(ib)
                a0 = apl.tile([128, HW], F32, tag="a0", name="a0")
                a1 = apl.tile([65, HW], F32, tag="a1", name="a1")
                nc.gpsimd.memset(a1[64:65, :], 1.0)
                for h in range(HEADS):
                    if h == 2 and mid is not None:
                        mid()
                    if h == 0:
                        q = qk_tiles["qab"][0:64, :]
                        k = qk_tiles["kab"][0:64, :]
                        tp = (0, 0)
                    elif h == 1:
                        q = qk_tiles["qab"][64:128, :]
                        k = qk_tiles["kab"][64:128, :]
                        tp = (64, 0)
                    else:
                        q = qk_tiles["qc"][:]
                        k = qk_tiles["kc"][:]
                        tp = (0, 0)
                    pav = pp1.tile([65, HW], F32, tag="pav", bufs=2,
                                   name="pav")
                    for sc_i in range(8):
                        pqk = pp1.tile([128, HW], F32, tag="pqk", bufs=2,
                                       name="pqk")
                        for n in range(2):
                            nc.tensor.matmul(
                                pqk[:, 512 * n:512 * n + 512],
                                _r(k[:, 128 * sc_i:128 * sc_i + 128]),
                                _r(q[:, 512 * n:512 * n + 512]),
                                start=True, stop=True, tile_position=tp)
                        eT = ep.tile([128, HW], F32, tag="eT", name="eT")
                        nc.scalar.activation(_r(eT[:]), pqk[:], AF.Exp)
                        for n in range(2):
                            nc.tensor.matmul(
                                pav[:, 512 * n:512 * n + 512],
                                _r(v_sb[:, sc_i, h, :]),
                                _r(eT[:, 512 * n:512 * n + 512]),
                                start=(sc_i == 0), stop=(sc_i == 7))
                    dnc = dp.tile([1, HW], F32, tag="dnc", name="dnc")
                    nc.vector.tensor_copy(dnc[:], pav[64:65, :])
                    dnr = dp.tile([1, HW], F32, tag="dnr", name="dnr")
                    nc.vector.reciprocal_approx_fast(dnr[:], dnc[:])
                    rbs = dp.tile([64, HW], F32, tag="rbs", name="rbs")
                    nc.gpsimd.partition_broadcast(rbs[:], dnr[:], channels=64)
                    a_sl = (a0[64 * h:64 * h + 64, :] if h < 2
                            else a1[0:64, :])
                    nc.vector.tensor_mul(_r(a_sl), pav[0:64, :], rbs[:])
                s1a[ib] = (x0, x1, a0, a1)

            def s1_tail(ib):
                x0, x1, a0, a1 = s1a.pop(ib)
                y0 = yp.tile([128, HW], F32, tag="y0", name="y0")
                y1 = yp.tile([64, HW], F32, tag="y1", name="y1")
                for (mb, mw, yt, xt) in ((0, 128, y0, x0), (128, 64, y1, x1)):
                    for n in range(2):
                        pp = pp1.tile([mw, 512], F32, tag="pqk", bufs=2,
                                      name="pp")
                        nc.tensor.matmul(pp[:], _r(wp1a[:, mb:mb + mw]),
                                         _r(a0[:, 512 * n:512 * n + 512]),
                                         start=True, stop=False)
                        nc.tensor.matmul(pp[:], _r(wp1b[:, mb:mb + mw]),
                                         _r(a1[:, 512 * n:512 * n + 512]),
                                         start=False, stop=True)
                        nc.vector.tensor_add(yt[:, 512 * n:512 * n + 512],
                                             pp[:],
                                             xt[:, 512 * n:512 * n + 512])
                dst = a2a_in[ib][:].rearrange("j c p -> c j p")
                nc.sync.dma_start(out=dst[0:128, :, :],
                                  in_=y0[:].rearrange("c (j p) -> c j p", j=8))
                nc.sync.dma_start(out=dst[128:192, :, :],
                                  in_=y1[:].rearrange("c (j p) -> c j p", j=8))
                if no_coll:
                    nc.sync.dma_start(out=a2a_out[ib][:], in_=a2a_in[ib][:])
                else:
                    nc.gpsimd.collective_compute(
                        "AllToAll", OP.bypass,
                        replica_groups=[list(range(N_CORES))],
                        ins=[a2a_in[ib][:].opt()],
                        outs=[a2a_out[ib][:].opt()],
                    )
                for bb_ in range(B):
                    s2_staging_i(bb_, ib)

            s1_load(0)
            s1_gn(0)
            s1_preB(0)
            for ib in range(NB1):
                if ib + 1 < NB1:
                    s1_load(ib + 1)
                    s1_gn(ib + 1)
                s1_heads(ib)
                if ib + 1 < NB1:
                    s1_preB(ib + 1)
                s1_tail(ib)
            pp_s1.__exit__(None, None, None)

            # ============================================== STAGE 2
            pp_s2 = tc.tile_pool(name="pp_s2", bufs=1, space="PSUM")
            pp2 = pp_s2.__enter__()


            s2n = {}

            def s2_preA(b, blk):
                x2Sa, x2Sb = s2s[b]
                po = 32 * blk
                x2a = x2Sa[:, :, po:po + 32].rearrange("c t l -> c l t")
                x2b = x2Sb[:, :, po:po + 32].rearrange("c t l -> c l t")

                sts2 = []
                for xt, p, tg in ((x2a, 128, "a"), (x2b, 64, "b")):
                    st = gp.tile([p, 64], F32, tag=f"st2{tg}", name=f"st2{tg}")
                    nc.vector.reduce_sum(st[:, 0:32], xt, axis=AX.X)
                    scr = npl.tile([p, 32, T], F32, tag=f"sq2{tg}", bufs=1,
                                   name=f"sq2{tg}")
                    nc.gpsimd.tensor_mul(scr[:], xt, xt)
                    nc.vector.reduce_sum(st[:, 32:64], scr[:], axis=AX.X)
                    sts2.append(st)
                sc2a, sc2b = gn_scale_bias(pp2, sts2[0], sts2[1], g2a, g2b,
                                           32, 1.0 / (GS * T), "mm2", 4)

                xn2a = npl.tile([128, 32 * T], F32, tag="xn0", name="xn2a")
                xn2b = npl.tile([65, 32 * T], F32, tag="xn1", name="xn2b")
                for xt, xnt, sct, p in ((x2a, xn2a, sc2a, 128),
                                        (x2b, xn2b, sc2b, 64)):
                    xn3 = xnt[0:p, :].rearrange("c (l t) -> c l t", t=T)
                    nc.vector.tensor_tensor(
                        _r(xn3), xt,
                        sct[:, 0:32, None].to_broadcast((p, 32, T)),
                        op=OP.mult)
                    nc.vector.tensor_tensor(
                        _r(xn3), xn3,
                        sct[:, 32:64, None].to_broadcast((p, 32, T)),
                        op=OP.add)
                nc.gpsimd.memset(xn2b[64:65, :], 1.0)
                s2n[(b, blk)] = (x2a, x2b, xn2a, xn2b)

            def s2_preB(b, blk):
                x2a, x2b, xn2a, xn2b = s2n.pop((b, blk))
                qk2 = {}
                for name, cb, mw in (("qab", 0, 128), ("kab", 128, 128),
                                     ("qc", 256, 64), ("kc", 320, 64)):
                    dst = qkp.tile([mw, 512], F32, tag=name, name=f"q2{name}")
                    pq = pp2.tile([mw, 512], F32, tag="mm2", bufs=4,
                                  name="pq2")
                    nc.tensor.matmul(pq[:], _r(wqk2a[:, cb:cb + mw]),
                                     _r(xn2a[:]), start=True, stop=False)
                    nc.tensor.matmul(pq[:], _r(wqk2b[:, cb:cb + mw]),
                                     _r(xn2b[:]), start=False, stop=True)
                    nc.scalar.copy(dst[:], pq[:])
                    qk2[name] = dst

                v2 = vp.tile([128, 4, 3, 65], F32, tag="v_sb", name="v2")
                nc.gpsimd.memset(v2[:, :, :, 64:65], 1.0)
                for o in range(4):
                    pv = pp2.tile([128, 192], F32, tag="mm2", bufs=4,
                                  name="pv2")
                    nc.tensor.matmul(pv[:], xn2a[:, 128 * o:128 * o + 128],
                                     wv2a[:], start=True, stop=False)
                    nc.tensor.matmul(pv[:], xn2b[:, 128 * o:128 * o + 128],
                                     wv2b[:], start=False, stop=True)
                    nc.scalar.copy(
                        v2[:, o, :, 0:64],
                        pv[:].rearrange("p (h c) -> p h c", h=3))
                s2[(b, blk)] = (x2a, x2b, qk2, v2)

            s2a = {}

            def s2_heads(b, blk):
                x2a, x2b, qk2, v2 = s2.pop((b, blk))
                a2_0 = apl.tile([128, 512], F32, tag="a0", name="a2_0")
                a2_1 = apl.tile([65, 512], F32, tag="a1", name="a2_1")
                nc.gpsimd.memset(a2_1[64:65, :], 1.0)
                for h in range(HEADS):
                    if h == 0:
                        q = qk2["qab"][0:64, :]
                        k = qk2["kab"][0:64, :]
                        tp = (0, 0)
                    elif h == 1:
                        q = qk2["qab"][64:128, :]
                        k = qk2["kab"][64:128, :]
                        tp = (64, 0)
                    else:
                        q = qk2["qc"][:]
                        k = qk2["kc"][:]
                        tp = (0, 0)
                    ps2 = pp2.tile([128, 512], F32, tag="ps2", bufs=2,
                                   name="ps2")
                    nc.tensor.matmul(ps2[:], _r(negu[:]), _r(ones1[:]),
                                     start=True, stop=False)
                    nc.tensor.matmul(ps2[:], _r(bks[:]), _r(bk4s[:]),
                                     start=False, stop=False)
                    for o in range(4):
                        nc.tensor.matmul(ps2[:, 128 * o:128 * o + 128],
                                         k[:, 128 * o:128 * o + 128],
                                         q[:, 128 * o:128 * o + 128],
                                         start=False, stop=(o == 3),
                                         tile_position=tp)
                    eT2 = ep.tile([128, 512], F32, tag="eT", name="eT2")
                    nc.scalar.activation(eT2[:], ps2[:], AF.Exp)
                    pav2 = pp2.tile([65, 512], F32, tag="pav2", bufs=2,
                                    name="pav2")
                    for o in range(4):
                        nc.tensor.matmul(pav2[:, 128 * o:128 * o + 128],
                                         v2[:, o, h, :],
                                         eT2[:, 128 * o:128 * o + 128],
                                         start=(o == 0), stop=(o == 3))
                    dn2c = dp.tile([1, 512], F32, tag="dnc", name="dn2c")
                    nc.scalar.copy(dn2c[:], pav2[64:65, :])
                    dn2r = dp.tile([1, 512], F32, tag="dnr", name="dn2r")
                    nc.vector.reciprocal_approx_fast(dn2r[:], dn2c[:])
                    rbs2 = dp.tile([64, 512], F32, tag="rbs", name="rbs2")
                    nc.gpsimd.partition_broadcast(rbs2[:], dn2r[:],
                                                  channels=64)
                    a_sl = (a2_0[64 * h:64 * h + 64, :] if h < 2
                            else a2_1[0:64, :])
                    nc.vector.tensor_mul(_r(a_sl), pav2[0:64, :], rbs2[:])
                s2a[(b, blk)] = (x2a, x2b, a2_0, a2_1)

            def s2_tail(b, blk):
                po = 32 * blk
                x2a, x2b, a2_0, a2_1 = s2a.pop((b, blk))
                for (mb, mw, xt, c0, c1) in ((0, 128, x2a, 0, 128),
                                             (128, 64, x2b, 128, 192)):
                    pp = pp2.tile([mw, 512], F32, tag="mm2", bufs=4,
                                  name="pp2")
                    nc.tensor.matmul(pp[:], _r(wp2a[:, mb:mb + mw]),
                                     _r(a2_0[:]), start=True, stop=False)
                    nc.tensor.matmul(pp[:], _r(wp2b[:, mb:mb + mw]),
                                     _r(a2_1[:]), start=False, stop=True)
                    y2 = yp.tile([mw, 32, T], F32,
                                 tag=("y0" if mb == 0 else "y1"),
                                 name="y2")
                    nc.vector.tensor_add(
                        y2[:], pp[:].rearrange("c (l t) -> c l t", t=T), xt)
                    nc.sync.dma_start(out=z_out[b, c0:c1, po:po + 32, :],
                                      in_=y2[:])

            blocks = [(b, blk) for b in range(B) for blk in range(4)]
            s2_preA(*blocks[0])
            s2_preB(*blocks[0])
            for i, bb in enumerate(blocks):
                if i + 1 < len(blocks):
                    s2_preA(*blocks[i + 1])
                s2_heads(*bb)
                if i + 1 < len(blocks):
                    s2_preB(*blocks[i + 1])
                s2_tail(*bb)
            pp_s2.__exit__(None, None, None)

    nc.compile()
    return nc


_NC_LOCK = threading.Lock()
_NC_CACHE = {}


def _get_nc():
    with _NC_LOCK:
        if "nc" not in _NC_CACHE:
            _NC_CACHE["nc"] = build_nc()
        return _NC_CACHE["nc"]


def kernel(x, norm_g, norm_b, qkv_w, qkv_b, proj_w, proj_b,
           normt_g, normt_b, qkvt_w, qkvt_b, projt_w, projt_b):
    x = np.asarray(x, np.float32)
    nc = _get_nc()

    xs = np.ascontiguousarray(
        np.transpose(x, (0, 2, 1, 3, 4)).reshape(B * T, C, HW))
    shared = {
        "wqk1": _pack_qkv(np.asarray(qkv_w, np.float32),
                          np.asarray(qkv_b, np.float32), QKSCALE),
        "wv1": _pack_v(np.asarray(qkv_w, np.float32),
                       np.asarray(qkv_b, np.float32)),
        "wp1": _pack_p(np.asarray(proj_w, np.float32),
                       np.asarray(proj_b, np.float32)),
        "gn1": np.stack([np.asarray(norm_g, np.float32),
                         np.asarray(norm_b, np.float32)], 1),
        "wqk2": _pack_qkv(np.asarray(qkvt_w, np.float32),
                          np.asarray(qkvt_b, np.float32), QKSCALE),
        "wv2": _pack_v(np.asarray(qkvt_w, np.float32),
                       np.asarray(qkvt_b, np.float32)),
        "wp2": _pack_p(np.asarray(projt_w, np.float32),
                       np.asarray(projt_b, np.float32)),
        "gn2": np.stack([np.asarray(normt_g, np.float32),
                         np.asarray(normt_b, np.float32)], 1),
    }
    in_maps = []
    for core in range(N_CORES):
        m = dict(shared)
        m["x_in"] = np.ascontiguousarray(xs[NB1 * core:NB1 * core + NB1])
        in_maps.append(m)

    res = run_bass_kernel_spmd(nc, in_maps, core_ids=list(range(N_CORES)))

    # z_out per core: [B, C, 128, T], hw = 128*core + p
    zhw = np.concatenate([res.results[c]["z_out"] for c in range(N_CORES)],
                         axis=2)                      # [B, C, 1024, T]
    z = np.transpose(zhw, (0, 1, 3, 2)).reshape(B, C, T, 32, 32)
    return np.ascontiguousarray(z)


# revision 50
# speedup vs baseline: 1.0114x; 1.0009x over previous
"""Fused spatial+temporal AttentionBlock kernel for 8 Trainium2 NeuronCores.

Sharding: stage 1 (spatial attention, batch (b t) = 32) data-parallel, 4
batches/core; stage 2 (temporal attention, batch (b h w) = 2048)
data-parallel over hw, 128 positions/core; 4 AllToAll collectives between
stages (first three hidden under stage-1 compute).

Attention uses the scoresT [s, t] orientation: softmax denominator comes from
a ones-column in the AV matmul; exp skips max-subtraction (scores bounded,
fp32-safe, softmax shift-invariant).  Large matmuls run as float32r (1
cycle/row vs 4 for fp32).  GroupNorm rstd is computed on the vector engine
(bit-trick + 2 Newton steps) so ACT only ever needs the exp table set.
Both stage loops are software-pipelined (prefix of iteration k+1 emitted
before body of k) since engines execute their streams in order.
"""

import os
import sys
import threading

sys.path.insert(0, '/opt/trn_rl_repo')
sys.path.insert(0, '/opt/trn_rl_repo/concourse')

import numpy as np

import concourse.bass as bass
import concourse.tile as tile
from concourse import bacc, mybir
from concourse.bass_utils import run_bass_kernel_spmd

F32 = mybir.dt.float32
F32R = mybir.dt.float32r
I32 = mybir.dt.int32
AF = mybir.ActivationFunctionType
OP = mybir.AluOpType
AX = mybir.AxisListType


def _r(ap):
    return ap.bitcast(F32R)


N_CORES = 8
C = 192
B = 2
T = 16
HW = 1024
NB1 = 4
PHW = 128
HEADS = 3
CH = 64
NGRP = 32
GS = 6
EPS = 1e-5
QKSCALE = CH ** -0.25
BIG = 60.0


def _consts():
    G0 = np.zeros((128, 32), np.float32)
    G1 = np.zeros((64, 32), np.float32)
    for c in range(128):
        G0[c, c // GS] = 1.0
    for c in range(128, 192):
        G1[c - 128, c // GS] = 1.0
    E0 = G0.T.copy()
    E1 = G1.T.copy()
    sq = np.sqrt(BIG).astype(np.float32)
    bk = np.zeros((8, 128), np.float32)
    for r in range(8):
        bk[r, 16 * r:16 * r + 16] = sq
    bk4 = np.tile(bk, (1, 4))
    return G0, G1, E0, E1, bk, bk4


def _pack_qkv(w, b, scale):
    qs = [w[192 * h:192 * h + 64] for h in range(3)]
    ks = [w[192 * h + 64:192 * h + 128] for h in range(3)]
    qb = [b[192 * h:192 * h + 64] for h in range(3)]
    kb = [b[192 * h + 64:192 * h + 128] for h in range(3)]
    rows = np.concatenate([qs[0], qs[1], ks[0], ks[1], qs[2], ks[2]], 0) * scale
    brow = np.concatenate([qb[0], qb[1], kb[0], kb[1], qb[2], kb[2]], 0) * scale
    return np.concatenate([rows.T, brow[None, :]], 0).astype(np.float32)


def _pack_v(w, b):
    vs = [w[192 * h + 128:192 * h + 192] for h in range(3)]
    vb = [b[192 * h + 128:192 * h + 192] for h in range(3)]
    rows = np.concatenate(vs, 0)
    brow = np.concatenate(vb, 0)
    out = np.concatenate([rows.T, brow[None, :]], 0).astype(np.float32)
    # pad to 256 output columns: moving-dim >= 256 runs fp32r at 1 cycle/row
    return np.concatenate([out, np.zeros((193, 64), np.float32)], 1)


def _pack_p(w, b):
    return np.concatenate([w.T, b[None, :]], 0).astype(np.float32)


def build_nc():
    nc = bacc.Bacc("TRN2", target_bir_lowering=False, debug=False,
                   num_devices=N_CORES)

    x_in = nc.dram_tensor("x_in", [NB1, C, HW], F32, kind="ExternalInput").ap()
    wqk1 = nc.dram_tensor("wqk1", [193, 384], F32, kind="ExternalInput").ap()
    wv1 = nc.dram_tensor("wv1", [193, 256], F32, kind="ExternalInput").ap()
    wp1 = nc.dram_tensor("wp1", [193, 192], F32, kind="ExternalInput").ap()
    gn1 = nc.dram_tensor("gn1", [C, 2], F32, kind="ExternalInput").ap()
    wqk2 = nc.dram_tensor("wqk2", [193, 384], F32, kind="ExternalInput").ap()
    wv2 = nc.dram_tensor("wv2", [193, 256], F32, kind="ExternalInput").ap()
    wp2 = nc.dram_tensor("wp2", [193, 192], F32, kind="ExternalInput").ap()
    gn2 = nc.dram_tensor("gn2", [C, 2], F32, kind="ExternalInput").ap()
    z_out = nc.dram_tensor("z_out", [B, C, PHW, T], F32,
                           kind="ExternalOutput").ap()

    G0n, G1n, E0n, E1n, bkn, bk4n = _consts()
    G0d = nc.inline_tensor(G0n, "G0c").ap()
    G1d = nc.inline_tensor(G1n, "G1c").ap()
    E0d = nc.inline_tensor(E0n, "E0c").ap()
    E1d = nc.inline_tensor(E1n, "E1c").ap()
    bkd = nc.inline_tensor(bkn, "bkc").ap()
    bk4d = nc.inline_tensor(bk4n, "bk4c").ap()
    magicd = nc.inline_tensor(np.full((32, 32), 0x5F3759DF, np.int32),
                              "magicc").ap()

    no_coll = os.environ.get("NO_COLLECTIVE", "0") == "1"

    with tile.TileContext(nc) as tc:
        with (
            tc.tile_pool(name="wts", bufs=1) as wts,
            tc.tile_pool(name="dram", bufs=1, space="DRAM") as dram,
            tc.tile_pool(name="xp", bufs=2) as xp,
            tc.tile_pool(name="np_", bufs=2) as npl,
            tc.tile_pool(name="gp", bufs=2) as gp,
            tc.tile_pool(name="qkp", bufs=2) as qkp,
            tc.tile_pool(name="vp", bufs=2) as vp,
            tc.tile_pool(name="ep", bufs=2) as ep,
            tc.tile_pool(name="ap_", bufs=2) as apl,
            tc.tile_pool(name="yp", bufs=2) as yp,
            tc.tile_pool(name="dp", bufs=2) as dp,
        ):
            def wtile(srcap, p, f, tag, rnd=False, dt=F32):
                t = wts.tile([p, f], dt, tag=tag, name=tag)
                if rnd:
                    nc.sync.dma_start(out=_r(t[:]), in_=_r(srcap))
                else:
                    nc.sync.dma_start(out=t[:], in_=srcap)
                return t

            wqk1a = wtile(wqk1[0:128, :], 128, 384, "wqk1a", rnd=True)
            wqk1b = wtile(wqk1[128:193, :], 65, 384, "wqk1b", rnd=True)
            wv1a = wtile(wv1[0:128, :], 128, 256, "wv1a", rnd=True)
            wv1b = wtile(wv1[128:193, :], 65, 256, "wv1b", rnd=True)
            wp1a = wtile(wp1[0:128, :], 128, 192, "wp1a", rnd=True)
            wp1b = wtile(wp1[128:193, :], 65, 192, "wp1b", rnd=True)
            wqk2a = wtile(wqk2[0:128, :], 128, 384, "wqk2a", rnd=True)
            wqk2b = wtile(wqk2[128:193, :], 65, 384, "wqk2b", rnd=True)
            wv2a = wtile(wv2[0:128, :], 128, 256, "wv2a", rnd=True)
            wv2b = wtile(wv2[128:193, :], 65, 256, "wv2b", rnd=True)
            wp2a = wtile(wp2[0:128, :], 128, 192, "wp2a", rnd=True)
            wp2b = wtile(wp2[128:193, :], 65, 192, "wp2b", rnd=True)
            g1a = wtile(gn1[0:128, :], 128, 2, "g1a")
            g1b = wtile(gn1[128:192, :], 64, 2, "g1b")
            g2a = wtile(gn2[0:128, :], 128, 2, "g2a")
            g2b = wtile(gn2[128:192, :], 64, 2, "g2b")
            G0s = wtile(G0d, 128, 32, "G0s")
            G1s = wtile(G1d, 64, 32, "G1s")
            E0s = wtile(E0d, 32, 128, "E0s")
            E1s = wtile(E1d, 32, 64, "E1s")
            bks = wtile(bkd, 8, 128, "bks", rnd=True)
            bk4s = wtile(bk4d, 8, 512, "bk4s", rnd=True)
            magi = wtile(magicd, 32, 32, "magi", dt=I32)
            negu = wts.tile([1, 128], F32, tag="negu", name="negu")
            nc.vector.memset(negu[:], -BIG)
            ones1 = wts.tile([1, 512], F32, tag="ones1", name="ones1")
            nc.vector.memset(ones1[:], 1.0)

            a2a_in = [dram.tile([N_CORES, C, PHW], F32, tag=f"a2ai{i}",
                                name=f"a2ai{i}") for i in range(NB1)]
            a2a_out = [dram.tile([N_CORES, C, PHW], F32, tag=f"a2ao{i}",
                                 name=f"a2ao{i}") for i in range(NB1)]

            def gn_scale_bias(ppmm, st_a, st_b, ga, gb, nb, inv_cnt, ptag, pbufs):
                pg = ppmm.tile([32, 2 * nb], F32, tag=ptag, bufs=pbufs,
                               name="pg")
                nc.tensor.matmul(pg[:], G0s[:], st_a[:], start=True,
                                 stop=False)
                nc.tensor.matmul(pg[:], G1s[:], st_b[:], start=False,
                                 stop=True)
                grp = gp.tile([32, 2 * nb], F32, tag="grp", name="grp")
                tmp = gp.tile([32, 2 * nb], F32, tag="gtmp", name="gtmp")
                mu = grp[:, 0:nb]
                nc.vector.tensor_scalar_mul(mu, pg[:, 0:nb], inv_cnt)
                nc.vector.tensor_scalar_mul(tmp[:, 0:nb], pg[:, nb:2 * nb],
                                            inv_cnt)
                nc.vector.tensor_mul(tmp[:, nb:2 * nb], mu, mu)
                nc.vector.tensor_sub(tmp[:, 0:nb], tmp[:, 0:nb],
                                     tmp[:, nb:2 * nb])
                # rstd = rsqrt(var + eps): bit-trick seed + 2 Newton steps
                xv = gp.tile([32, nb], F32, tag="rsx", name="rsx")
                yv = gp.tile([32, nb], F32, tag="rsy", name="rsy")
                tv = gp.tile([32, nb], F32, tag="rst", name="rst")
                nc.vector.tensor_scalar_add(xv[:], tmp[:, 0:nb], EPS)
                nc.vector.tensor_scalar(yv[:].bitcast(I32),
                                        xv[:].bitcast(I32), 1, None,
                                        op0=OP.arith_shift_right)
                nc.vector.tensor_sub(yv[:].bitcast(I32), magi[:, 0:nb],
                                     yv[:].bitcast(I32))
                for _ in range(2):
                    nc.vector.tensor_mul(tv[:], yv[:], yv[:])
                    nc.vector.tensor_mul(tv[:], tv[:], xv[:])
                    nc.vector.tensor_scalar(tv[:], tv[:], -0.5, 1.5,
                                            op0=OP.mult, op1=OP.add)
                    nc.vector.tensor_mul(yv[:], yv[:], tv[:])
                nc.vector.tensor_copy(grp[:, nb:2 * nb], yv[:])
                out = []
                for Es, gch, p in ((E0s, ga, 128), (E1s, gb, 64)):
                    psc = ppmm.tile([p, 2 * nb], F32, tag=ptag, bufs=pbufs,
                                    name="psc")
                    nc.tensor.matmul(psc[:], Es[:], grp[:], start=True,
                                     stop=True)
                    sc = gp.tile([p, 2 * nb], F32, tag=f"sc{p}",
                                 name=f"sc{p}")
                    nc.vector.tensor_scalar_mul(sc[:, 0:nb], psc[:, nb:2 * nb],
                                                gch[:, 0:1])
                    nc.vector.tensor_mul(sc[:, nb:2 * nb], psc[:, 0:nb],
                                         sc[:, 0:nb])
                    nc.vector.tensor_tensor(
                        sc[:, nb:2 * nb],
                        gch[:, 1:2].to_broadcast((p, nb)),
                        sc[:, nb:2 * nb], op=OP.subtract)
                    out.append(sc)
                return out

            s2s = {}
            s2 = {}

            def s2_staging_i(b, i):
                if b not in s2s:
                    x2Sa = xp.tile([128, T, PHW], F32, tag="x2Sa", bufs=2,
                                   name="x2Sa")
                    x2Sb = xp.tile([64, T, PHW], F32, tag="x2Sb", bufs=2,
                                   name="x2Sb")
                    s2s[b] = (x2Sa, x2Sb)
                x2Sa, x2Sb = s2s[b]
                srcv = a2a_out[i][:].rearrange("s c p -> c p s")
                for xt, c0, c1 in ((x2Sa, 0, 128), (x2Sb, 128, 192)):
                    for j in range(4):
                        nc.sync.dma_start(
                            out=xt[:, 4 * j + i, :],
                            in_=srcv[c0:c1, :, 4 * b + j])

            # ============================================== STAGE 1
            pp_s1 = tc.tile_pool(name="pp_s1", bufs=1, space="PSUM")
            pp1 = pp_s1.__enter__()

            s1 = {}
            s1n = {}
            s1st = {}

            def s1_load(ib):
                x0 = xp.tile([128, HW], F32, tag="x0", name="x0")
                x1 = xp.tile([64, HW], F32, tag="x1", name="x1")
                nc.sync.dma_start(out=x0[:], in_=x_in[ib, 0:128, :])
                nc.sync.dma_start(out=x1[:], in_=x_in[ib, 128:192, :])

                sts = []
                for xt, p, tg in ((x0, 128, "a"), (x1, 64, "b")):
                    stat = gp.tile([p, 2, 6], F32, tag=f"bst{tg}",
                                   name=f"bst{tg}")
                    nc.vector.bn_stats(stat[:, 0, :], xt[:, 0:512])
                    nc.vector.bn_stats(stat[:, 1, :], xt[:, 512:1024])
                    mv = gp.tile([p, 2], F32, tag=f"mv{tg}", name=f"mv{tg}")
                    nc.vector.bn_aggr(mv[:], stat[:])
                    st = gp.tile([p, 2], F32, tag=f"st{tg}", name=f"st{tg}")
                    nc.vector.tensor_copy(st[:, 0:1], mv[:, 0:1])
                    nc.vector.tensor_mul(st[:, 1:2], mv[:, 0:1], mv[:, 0:1])
                    nc.vector.tensor_add(st[:, 1:2], st[:, 1:2], mv[:, 1:2])
                    sts.append(st)
                s1st[ib] = (x0, x1, sts)

            def s1_gn(ib):
                x0, x1, sts = s1st.pop(ib)
                sc0, sc1 = gn_scale_bias(pp1, sts[0], sts[1], g1a, g1b, 1,
                                         1.0 / GS, "pqk", 2)

                xn0 = npl.tile([128, HW], F32, tag="xn0", name="xn0")
                xn1 = npl.tile([65, HW], F32, tag="xn1", name="xn1")
                nc.vector.tensor_scalar(_r(xn0[:]), x0[:], sc0[:, 0:1],
                                        sc0[:, 1:2], op0=OP.mult, op1=OP.add)
                nc.vector.tensor_scalar(_r(xn1[0:64, :]), x1[:], sc1[:, 0:1],
                                        sc1[:, 1:2], op0=OP.mult, op1=OP.add)
                nc.gpsimd.memset(xn1[64:65, :], 1.0)
                s1n[ib] = (x0, x1, xn0, xn1)

            def s1_preB(ib):
                x0, x1, xn0, xn1 = s1n.pop(ib)
                qk_tiles = {}
                for name, cb, mw in (("qab", 0, 128), ("kab", 128, 128),
                                     ("qc", 256, 64), ("kc", 320, 64)):
                    dst = qkp.tile([mw, HW], F32, tag=name, name=name)
                    for n in range(2):
                        pq = pp1.tile([mw, 512], F32, tag="pqk", bufs=2,
                                      name="pq")
                        nc.tensor.matmul(pq[:], _r(wqk1a[:, cb:cb + mw]),
                                         _r(xn0[:, 512 * n:512 * n + 512]),
                                         start=True, stop=False)
                        nc.tensor.matmul(pq[:], _r(wqk1b[:, cb:cb + mw]),
                                         _r(xn1[:, 512 * n:512 * n + 512]),
                                         start=False, stop=True)
                        nc.scalar.copy(
                            _r(dst[:, 512 * n:512 * n + 512]), pq[:])
                    qk_tiles[name] = dst

                v_sb = vp.tile([128, 8, 3, 65], F32, tag="v_sb", name="v_sb")
                nc.gpsimd.memset(v_sb[:, :, :, 64:65], 1.0)
                for lc in range(8):
                    pv = pp1.tile([128, 256], F32, tag="pqk", bufs=2,
                                  name="pv")
                    nc.tensor.matmul(pv[:],
                                     _r(xn0[:, 128 * lc:128 * lc + 128]),
                                     _r(wv1a[:]), start=True, stop=False)
                    nc.tensor.matmul(pv[:],
                                     _r(xn1[:, 128 * lc:128 * lc + 128]),
                                     _r(wv1b[:]), start=False, stop=True)
                    nc.vector.tensor_copy(
                        _r(v_sb[:, lc, :, 0:64]),
                        pv[:, 0:192].rearrange("p (h c) -> p h c", h=3))
                s1[ib] = (x0, x1, qk_tiles, v_sb)

            s1a = {}

            def s1_heads(ib, mid=None):
                x0, x1, qk_tiles, v_sb = s1.pop(ib)
                a0 = apl.tile([128, HW], F32, tag="a0", name="a0")
                a1 = apl.tile([65, HW], F32, tag="a1", name="a1")
                nc.gpsimd.memset(a1[64:65, :], 1.0)
                for h in range(HEADS):
                    if h == 2 and mid is not None:
                        mid()
                    if h == 0:
                        q = qk_tiles["qab"][0:64, :]
                        k = qk_tiles["kab"][0:64, :]
                        tp = (0, 0)
                    elif h == 1:
                        q = qk_tiles["qab"][64:128, :]
                        k = qk_tiles["kab"][64:128, :]
                        tp = (64, 0)
                    else:
                        q = qk_tiles["qc"][:]
                        k = qk_tiles["kc"][:]
                        tp = (0, 0)
                    pav = pp1.tile([65, HW], F32, tag="pav", bufs=2,
                                   name="pav")
                    for sc_i in range(8):
                        pqk = pp1.tile([128, HW], F32, tag="pqk", bufs=2,
                                       name="pqk")
                        for n in range(2):
                            nc.tensor.matmul(
                                pqk[:, 512 * n:512 * n + 512],
                                _r(k[:, 128 * sc_i:128 * sc_i + 128]),
                                _r(q[:, 512 * n:512 * n + 512]),
                                start=True, stop=True, tile_position=tp)
                        eT = ep.tile([128, HW], F32, tag="eT", name="eT")
                        nc.scalar.activation(_r(eT[:]), pqk[:], AF.Exp)
                        for n in range(2):
                            nc.tensor.matmul(
                                pav[:, 512 * n:512 * n + 512],
                                _r(v_sb[:, sc_i, h, :]),
                                _r(eT[:, 512 * n:512 * n + 512]),
                                start=(sc_i == 0), stop=(sc_i == 7))
                    dnc = dp.tile([1, HW], F32, tag="dnc", name="dnc")
                    nc.vector.tensor_copy(dnc[:], pav[64:65, :])
                    dnr = dp.tile([1, HW], F32, tag="dnr", name="dnr")
                    nc.vector.reciprocal_approx_fast(dnr[:], dnc[:])
                    rbs = dp.tile([64, HW], F32, tag="rbs", name="rbs")
                    nc.gpsimd.partition_broadcast(rbs[:], dnr[:], channels=64)
                    a_sl = (a0[64 * h:64 * h + 64, :] if h < 2
                            else a1[0:64, :])
                    nc.vector.tensor_mul(_r(a_sl), pav[0:64, :], rbs[:])
                s1a[ib] = (x0, x1, a0, a1)

            def s1_tail(ib):
                x0, x1, a0, a1 = s1a.pop(ib)
                y0 = yp.tile([128, HW], F32, tag="y0", name="y0")
                y1 = yp.tile([64, HW], F32, tag="y1", name="y1")
                for (mb, mw, yt, xt) in ((0, 128, y0, x0), (128, 64, y1, x1)):
                    for n in range(2):
                        pp = pp1.tile([mw, 512], F32, tag="pqk", bufs=2,
                                      name="pp")
                        nc.tensor.matmul(pp[:], _r(wp1a[:, mb:mb + mw]),
                                         _r(a0[:, 512 * n:512 * n + 512]),
                                         start=True, stop=False)
                        nc.tensor.matmul(pp[:], _r(wp1b[:, mb:mb + mw]),
                                         _r(a1[:, 512 * n:512 * n + 512]),
                                         start=False, stop=True)
                        nc.vector.tensor_add(yt[:, 512 * n:512 * n + 512],
                                             pp[:],
                                             xt[:, 512 * n:512 * n + 512])
                dst = a2a_in[ib][:].rearrange("j c p -> c j p")
                nc.sync.dma_start(out=dst[0:128, :, :],
                                  in_=y0[:].rearrange("c (j p) -> c j p", j=8))
                nc.sync.dma_start(out=dst[128:192, :, :],
                                  in_=y1[:].rearrange("c (j p) -> c j p", j=8))
                if no_coll:
                    nc.sync.dma_start(out=a2a_out[ib][:], in_=a2a_in[ib][:])
                else:
                    nc.gpsimd.collective_compute(
                        "AllToAll", OP.bypass,
                        replica_groups=[list(range(N_CORES))],
                        ins=[a2a_in[ib][:].opt()],
                        outs=[a2a_out[ib][:].opt()],
                    )
                for bb_ in range(B):
                    s2_staging_i(bb_, ib)

            s1_load(0)
            s1_gn(0)
            s1_preB(0)
            for ib in range(NB1):
                if ib + 1 < NB1:
                    s1_load(ib + 1)
                    s1_gn(ib + 1)
                s1_heads(ib)
                if ib + 1 < NB1:
                    s1_preB(ib + 1)
                s1_tail(ib)
            pp_s1.__exit__(None, None, None)

            # ============================================== STAGE 2
            pp_s2 = tc.tile_pool(name="pp_s2", bufs=1, space="PSUM")
            pp2 = pp_s2.__enter__()


            s2n = {}

            def s2_preA(b, blk):
                x2Sa, x2Sb = s2s[b]
                po = 32 * blk
                x2a = x2Sa[:, :, po:po + 32].rearrange("c t l -> c l t")
                x2b = x2Sb[:, :, po:po + 32].rearrange("c t l -> c l t")

                sts2 = []
                for xt, p, tg in ((x2a, 128, "a"), (x2b, 64, "b")):
                    st = gp.tile([p, 64], F32, tag=f"st2{tg}", name=f"st2{tg}")
                    nc.vector.reduce_sum(st[:, 0:32], xt, axis=AX.X)
                    scr = npl.tile([p, 32, T], F32, tag=f"sq2{tg}", bufs=1,
                                   name=f"sq2{tg}")
                    nc.gpsimd.tensor_mul(scr[:], xt, xt)
                    nc.vector.reduce_sum(st[:, 32:64], scr[:], axis=AX.X)
                    sts2.append(st)
                sc2a, sc2b = gn_scale_bias(pp2, sts2[0], sts2[1], g2a, g2b,
                                           32, 1.0 / (GS * T), "mm2", 4)

                xn2a = npl.tile([128, 32 * T], F32, tag="xn0", name="xn2a")
                xn2b = npl.tile([65, 32 * T], F32, tag="xn1", name="xn2b")
                for xt, xnt, sct, p in ((x2a, xn2a, sc2a, 128),
                                        (x2b, xn2b, sc2b, 64)):
                    xn3 = xnt[0:p, :].rearrange("c (l t) -> c l t", t=T)
                    nc.vector.tensor_tensor(
                        _r(xn3), xt,
                        sct[:, 0:32, None].to_broadcast((p, 32, T)),
                        op=OP.mult)
                    nc.vector.tensor_tensor(
                        _r(xn3), xn3,
                        sct[:, 32:64, None].to_broadcast((p, 32, T)),
                        op=OP.add)
                nc.gpsimd.memset(xn2b[64:65, :], 1.0)
                s2n[(b, blk)] = (x2a, x2b, xn2a, xn2b)

            def s2_preB(b, blk):
                x2a, x2b, xn2a, xn2b = s2n.pop((b, blk))
                qk2 = {}
                for name, cb, mw in (("qab", 0, 128), ("kab", 128, 128),
                                     ("qc", 256, 64), ("kc", 320, 64)):
                    dst = qkp.tile([mw, 512], F32, tag=name, name=f"q2{name}")
                    pq = pp2.tile([mw, 512], F32, tag="mm2", bufs=4,
                                  name="pq2")
                    nc.tensor.matmul(pq[:], _r(wqk2a[:, cb:cb + mw]),
                                     _r(xn2a[:]), start=True, stop=False)
                    nc.tensor.matmul(pq[:], _r(wqk2b[:, cb:cb + mw]),
                                     _r(xn2b[:]), start=False, stop=True)
                    nc.scalar.copy(dst[:], pq[:])
                    qk2[name] = dst

                v2 = vp.tile([128, 4, 3, 65], F32, tag="v_sb", name="v2")
                nc.gpsimd.memset(v2[:, :, :, 64:65], 1.0)
                for o in range(4):
                    pv = pp2.tile([128, 256], F32, tag="mm2", bufs=4,
                                  name="pv2")
                    nc.tensor.matmul(pv[:],
                                     _r(xn2a[:, 128 * o:128 * o + 128]),
                                     _r(wv2a[:]), start=True, stop=False)
                    nc.tensor.matmul(pv[:],
                                     _r(xn2b[:, 128 * o:128 * o + 128]),
                                     _r(wv2b[:]), start=False, stop=True)
                    nc.scalar.copy(
                        v2[:, o, :, 0:64],
                        pv[:, 0:192].rearrange("p (h c) -> p h c", h=3))
                s2[(b, blk)] = (x2a, x2b, qk2, v2)

            s2a = {}

            def s2_heads(b, blk):
                x2a, x2b, qk2, v2 = s2.pop((b, blk))
                a2_0 = apl.tile([128, 512], F32, tag="a0", name="a2_0")
                a2_1 = apl.tile([65, 512], F32, tag="a1", name="a2_1")
                nc.gpsimd.memset(a2_1[64:65, :], 1.0)
                for h in range(HEADS):
                    if h == 0:
                        q = qk2["qab"][0:64, :]
                        k = qk2["kab"][0:64, :]
                        tp = (0, 0)
                    elif h == 1:
                        q = qk2["qab"][64:128, :]
                        k = qk2["kab"][64:128, :]
                        tp = (64, 0)
                    else:
                        q = qk2["qc"][:]
                        k = qk2["kc"][:]
                        tp = (0, 0)
                    ps2 = pp2.tile([128, 512], F32, tag="ps2", bufs=2,
                                   name="ps2")
                    nc.tensor.matmul(ps2[:], _r(negu[:]), _r(ones1[:]),
                                     start=True, stop=False)
                    nc.tensor.matmul(ps2[:], _r(bks[:]), _r(bk4s[:]),
                                     start=False, stop=False)
                    for o in range(4):
                        nc.tensor.matmul(ps2[:, 128 * o:128 * o + 128],
                                         k[:, 128 * o:128 * o + 128],
                                         q[:, 128 * o:128 * o + 128],
                                         start=False, stop=(o == 3),
                                         tile_position=tp)
                    eT2 = ep.tile([128, 512], F32, tag="eT", name="eT2")
                    nc.scalar.activation(eT2[:], ps2[:], AF.Exp)
                    pav2 = pp2.tile([65, 512], F32, tag="pav2", bufs=2,
                                    name="pav2")
                    for o in range(4):
                        nc.tensor.matmul(pav2[:, 128 * o:128 * o + 128],
                                         v2[:, o, h, :],
                                         eT2[:, 128 * o:128 * o + 128],
                                         start=(o == 0), stop=(o == 3))
                    dn2c = dp.tile([1, 512], F32, tag="dnc", name="dn2c")
                    nc.scalar.copy(dn2c[:], pav2[64:65, :])
                    dn2r = dp.tile([1, 512], F32, tag="dnr", name="dn2r")
                    nc.vector.reciprocal_approx_fast(dn2r[:], dn2c[:])
                    rbs2 = dp.tile([64, 512], F32, tag="rbs", name="rbs2")
                    nc.gpsimd.partition_broadcast(rbs2[:], dn2r[:],
                                                  channels=64)
                    a_sl = (a2_0[64 * h:64 * h + 64, :] if h < 2
                            else a2_1[0:64, :])
                    nc.vector.tensor_mul(_r(a_sl), pav2[0:64, :], rbs2[:])
                s2a[(b, blk)] = (x2a, x2b, a2_0, a2_1)

            def s2_tail(b, blk):
                po = 32 * blk
                x2a, x2b, a2_0, a2_1 = s2a.pop((b, blk))
                for (mb, mw, xt, c0, c1) in ((0, 128, x2a, 0, 128),
                                             (128, 64, x2b, 128, 192)):
                    pp = pp2.tile([mw, 512], F32, tag="mm2", bufs=4,
                                  name="pp2")
                    nc.tensor.matmul(pp[:], _r(wp2a[:, mb:mb + mw]),
                                     _r(a2_0[:]), start=True, stop=False)
                    nc.tensor.matmul(pp[:], _r(wp2b[:, mb:mb + mw]),
                                     _r(a2_1[:]), start=False, stop=True)
                    y2 = yp.tile([mw, 32, T], F32,
                                 tag=("y0" if mb == 0 else "y1"),
                                 name="y2")
                    nc.vector.tensor_add(
                        y2[:], pp[:].rearrange("c (l t) -> c l t", t=T), xt)
                    nc.sync.dma_start(out=z_out[b, c0:c1, po:po + 32, :],
                                      in_=y2[:])

            blocks = [(b, blk) for b in range(B) for blk in range(4)]
            s2_preA(*blocks[0])
            s2_preB(*blocks[0])
            for i, bb in enumerate(blocks):
                if i + 1 < len(blocks):
                    s2_preA(*blocks[i + 1])
                s2_heads(*bb)
                if i + 1 < len(blocks):
                    s2_preB(*blocks[i + 1])
                s2_tail(*bb)
            pp_s2.__exit__(None, None, None)

    nc.compile()
    return nc


_NC_LOCK = threading.Lock()
_NC_CACHE = {}


def _get_nc():
    with _NC_LOCK:
        if "nc" not in _NC_CACHE:
            _NC_CACHE["nc"] = build_nc()
        return _NC_CACHE["nc"]


def kernel(x, norm_g, norm_b, qkv_w, qkv_b, proj_w, proj_b,
           normt_g, normt_b, qkvt_w, qkvt_b, projt_w, projt_b):
    x = np.asarray(x, np.float32)
    nc = _get_nc()

    xs = np.ascontiguousarray(
        np.transpose(x, (0, 2, 1, 3, 4)).reshape(B * T, C, HW))
    shared = {
        "wqk1": _pack_qkv(np.asarray(qkv_w, np.float32),
                          np.asarray(qkv_b, np.float32), QKSCALE),
        "wv1": _pack_v(np.asarray(qkv_w, np.float32),
                       np.asarray(qkv_b, np.float32)),
        "wp1": _pack_p(np.asarray(proj_w, np.float32),
                       np.asarray(proj_b, np.float32)),
        "gn1": np.stack([np.asarray(norm_g, np.float32),
                         np.asarray(norm_b, np.float32)], 1),
        "wqk2": _pack_qkv(np.asarray(qkvt_w, np.float32),
                          np.asarray(qkvt_b, np.float32), QKSCALE),
        "wv2": _pack_v(np.asarray(qkvt_w, np.float32),
                       np.asarray(qkvt_b, np.float32)),
        "wp2": _pack_p(np.asarray(projt_w, np.float32),
                       np.asarray(projt_b, np.float32)),
        "gn2": np.stack([np.asarray(normt_g, np.float32),
                         np.asarray(normt_b, np.float32)], 1),
    }
    in_maps = []
    for core in range(N_CORES):
        m = dict(shared)
        m["x_in"] = np.ascontiguousarray(xs[NB1 * core:NB1 * core + NB1])
        in_maps.append(m)

    res = run_bass_kernel_spmd(nc, in_maps, core_ids=list(range(N_CORES)))

    # z_out per core: [B, C, 128, T], hw = 128*core + p
    zhw = np.concatenate([res.results[c]["z_out"] for c in range(N_CORES)],
                         axis=2)                      # [B, C, 1024, T]
    z = np.transpose(zhw, (0, 1, 3, 2)).reshape(B, C, T, 32, 32)
    return np.ascontiguousarray(z)
